# revision 2
# baseline (speedup 1.0000x reference)
"""HGRN BitAttention Trainium2 kernel v2 (8-core SPMD, token-sharded).

Sharding: core c handles batch c//2, sequence half c%2 (1024 tokens).
The HGRN recurrence carry crosses the half boundary via a pair-AllReduce.

v2 layout strategy: ALL four projections run feature-major (weight tiles
stationary, quantized activations moving), so the gate/quantize stage needs
no PE transposes: per-token reductions over the feature (partition) dim are
done with ones-matmuls on the PE (sums) and gpsimd partition_all_reduce
(absmax).  Key algebra: for the o-quant, u = on*s = t * 127/max|t| where
t = gsc*silu(h) -- the g-norm factor and o-rmsnorm factor cancel there and
are folded into the output-projection drain scale instead.  Output is
written feature-major and transposed on the host.

BitLinear: activations quantized to ints in [-127,127], weights ternary;
both exact in bf16, so matmuls are exact-int bf16 with f32 PSUM.
"""

import numpy as np
import ml_dtypes

import concourse.bass as bass
import concourse.bacc as bacc
import concourse.mybir as mybir
import concourse.tile as tile
from concourse import bass_isa
from concourse.bass_utils import run_bass_kernel_spmd

F32 = mybir.dt.float32
BF16 = mybir.dt.bfloat16
F16 = mybir.dt.float16
I32 = mybir.dt.int32
AF = mybir.ActivationFunctionType
OP = mybir.AluOpType

B, L, D = 4, 2048, 2048
NCORES = 8
TPC = L // 2          # tokens per core = 1024
NTT = TPC // 128      # 8 token tiles per core
KT = D // 128         # 16 k tiles
MT = D // 128         # 16 m tiles
GCH = 2               # g-proj chunks (512 tokens)
OCH = 4               # gate/o-proj chunks (256 tokens)
OCW = TPC // OCH      # 256
EPS = 1e-5


def build_nc():
    nc = bacc.Bacc("TRN2", target_bir_lowering=False, debug=False,
                   num_devices=NCORES)

    x_d = nc.dram_tensor("x", [TPC, D], F32, kind="ExternalInput")
    # weights pre-tiled on host: flat [p, m, k, c] = WT[k*128+p, m*128+c]
    wift_d = nc.dram_tensor("wift", [128, MT * 2 * KT * 128], BF16,
                            kind="ExternalInput")   # [p, m, {i,f}, k, c]
    wgt_d = nc.dram_tensor("wgt", [128, MT * KT * 128], BF16,
                           kind="ExternalInput")    # [p, m, k, c]
    wot_d = nc.dram_tensor("wot", [128, MT * KT * 128], BF16,
                           kind="ExternalInput")    # [p, m, k, c]
    gwsc_d = nc.dram_tensor("gwsc", [128, MT], F32, kind="ExternalInput")
    me_d = nc.dram_tensor("mask_even", [128, 1], F32, kind="ExternalInput")
    mo_d = nc.dram_tensor("mask_odd", [128, 1], F32, kind="ExternalInput")
    rws_d = nc.dram_tensor("rws", [128, 4], F32, kind="ExternalInput")
    out_d = nc.dram_tensor("out", [D, TPC], F32, kind="ExternalOutput")

    with tile.TileContext(nc) as tc:
        with (
            tc.tile_pool(name="const", bufs=1) as cp,
            tc.tile_pool(name="hp", bufs=1) as hp,
            tc.tile_pool(name="rows", bufs=1) as rp,
            tc.tile_pool(name="dram", bufs=1, space="DRAM") as dram,
        ):
            # ---- constants ----
            me = cp.tile([128, 1], F32)
            nc.sync.dma_start(me[:], me_d.ap())
            mo = cp.tile([128, 1], F32)
            nc.sync.dma_start(mo[:], mo_d.ap())
            rws = cp.tile([128, 4], F32)
            nc.sync.dma_start(rws[:], rws_d.ap())
            rwsi, rwsf, rwsfn, rwso = (rws[:, i:i + 1] for i in range(4))
            gwsc = cp.tile([128, MT], F32)
            nc.sync.dma_start(gwsc[:], gwsc_d.ap())
            epsb = cp.tile([128, 1], F32)
            nc.vector.memset(epsb[:], EPS)
            ones1 = cp.tile([1, 128], F32)
            nc.vector.memset(ones1[:], 1.0)
            ones128 = cp.tile([128, 1], BF16)
            nc.vector.memset(ones128[:], 1.0)

            srec = cp.tile([128, NTT], F32)     # (1/s_x) per token tile col
            bnd = cp.tile([128, MT], F32)
            bnd2 = cp.tile([128, MT], F32)
            carried = cp.tile([128, MT], F32)
            carry_sb = cp.tile([128, MT], F32)

            hsB = hp.tile([128, MT * TPC], F32)           # h, feature-major

            # g (descaled) is staged via DRAM: written during the g-phase,
            # read back chunk-by-chunk in phase T.
            gsc_d = dram.tile([128, MT * TPC], F16)

            # pools freed mid-kernel (manual scopes; LIFO: free xqp after g,
            # then fcp after fixup)
            fc_ctx = tc.tile_pool(name="fcp", bufs=1)
            fcp = fc_ctx.__enter__()
            fcB = fcp.tile([128, MT * TPC], F16)          # cumprod(F)

            xq_ctx = tc.tile_pool(name="xqp", bufs=1)
            xqp = xq_ctx.__enter__()
            xqT = xqp.tile([128, KT * TPC], BF16)  # [d_in-major] quantized x
            xqT3 = xqT[:].rearrange("p (k t) -> p k t", k=KT)
            S = xqp.tile([128, TPC], F32)       # (1/s_x) broadcast, feat-major

            # =============== Phase X: normalize + quantize x ===============
            with (
                tc.tile_pool(name="xin", bufs=2) as xin,
                tc.tile_pool(name="xw", bufs=2) as xw,
                tc.tile_pool(name="psx", bufs=1, space="PSUM") as psx,
                tc.tile_pool(name="qip", bufs=1, space="PSUM") as qip,
            ):
                for tt in range(NTT):
                    xt = xin.tile([128, D], F32)
                    nc.sync.dma_start(xt[:], x_d.ap()[tt * 128:(tt + 1) * 128, :])
                    scr = xw.tile([128, D], BF16)
                    ssum = xw.tile([128, 1], F32)
                    nc.scalar.activation(scr[:], xt[:], AF.Square,
                                         accum_out=ssum[:])
                    std = xw.tile([128, 1], F32)
                    nc.scalar.activation(std[:], ssum[:], AF.Sqrt,
                                         bias=epsb[:], scale=1.0 / D)
                    rstd = xw.tile([128, 1], F32)
                    nc.vector.reciprocal(rstd[:], std[:])
                    # absmax(xn) = rstd * absmax(x)
                    mxa = xw.tile([128, 1], F32)
                    nc.vector.tensor_reduce(mxa[:], xt[:], mybir.AxisListType.X,
                                            OP.max, apply_absolute_value=True)
                    mxn = xw.tile([128, 1], F32)
                    nc.vector.tensor_tensor(mxn[:], mxa[:], rstd[:], OP.mult)
                    nc.vector.tensor_scalar_max(mxn[:], mxn[:], EPS)
                    nc.vector.tensor_scalar_mul(srec[:, tt:tt + 1], mxn[:],
                                                1.0 / 127.0)
                    sst = xw.tile([128, 1], F32)
                    nc.vector.reciprocal(sst[:], mxn[:])
                    # combined per-token scale: round(x * rstd * 127/mxn)
                    rs = xw.tile([128, 1], F32)
                    nc.vector.tensor_tensor(rs[:], rstd[:], sst[:], OP.mult)
                    nc.vector.tensor_scalar_mul(rs[:], rs[:], 127.0)
                    qi = qip.tile([128, D], I32)
                    nc.scalar.activation(qi[:], xt[:], AF.Identity, scale=rs[:])
                    qb = xw.tile([128, D], BF16)
                    nc.vector.tensor_copy(qb[:], qi[:])
                    nc.sync.dma_start_transpose(
                        xqT3[:, :, tt * 128:(tt + 1) * 128], qb[:])

                # S = broadcast of (1/s) to [128, TPC] feature-major
                srd = dram.tile([1, TPC], F32)
                nc.sync.dma_start(
                    srd[:].rearrange("o (t p) -> (o p) t", p=128), srec[:])
                srow = xw.tile([1, TPC], F32, name="srow")
                nc.sync.dma_start(srow[:], srd[:])
                for c in range(2):
                    pS = psx.tile([128, 512], F32)
                    nc.tensor.matmul(pS[:], ones1[:],
                                     srow[:, c * 512:(c + 1) * 512],
                                     start=True, stop=True)
                    nc.scalar.copy(S[:, c * 512:(c + 1) * 512], pS[:])

            # ========= Phase P: i/f projections + scans (feature-major) =========
            with (
                tc.tile_pool(name="wif", bufs=2) as wif,
                tc.tile_pool(name="pw", bufs=2) as pw,
                tc.tile_pool(name="psp", bufs=2, space="PSUM") as psp,
            ):
                for m in range(MT):
                    wm = wif.tile([128, 2 * KT * 128], BF16)
                    nc.sync.dma_start(
                        wm[:], wift_d.ap()[:, m * 2 * KT * 128:
                                           (m + 1) * 2 * KT * 128])
                    psi = psp.tile([128, TPC], F32, name="psi")
                    psf = psp.tile([128, TPC], F32, name="psf")
                    for k in range(KT):
                        wi_k = wm[:, k * 128:(k + 1) * 128]
                        wf_k = wm[:, (KT + k) * 128:(KT + k + 1) * 128]
                        st, sp = (k == 0), (k == KT - 1)
                        nc.tensor.matmul(psi[:, 0:512], wi_k,
                                         xqT[:, k * TPC:k * TPC + 512],
                                         start=st, stop=sp)
                        nc.tensor.matmul(psi[:, 512:TPC], wi_k,
                                         xqT[:, k * TPC + 512:(k + 1) * TPC],
                                         start=st, stop=sp)
                        nc.tensor.matmul(psf[:, 0:512], wf_k,
                                         xqT[:, k * TPC:k * TPC + 512],
                                         start=st, stop=sp)
                        nc.tensor.matmul(psf[:, 512:TPC], wf_k,
                                         xqT[:, k * TPC + 512:(k + 1) * TPC],
                                         start=st, stop=sp)
                    tmpf = pw.tile([128, TPC], F16, name="tmpf")
                    nc.vector.tensor_tensor(tmpf[:], psf[:], S[:], OP.mult)
                    tmpi = pw.tile([128, TPC], F16, name="tmpi")
                    nc.vector.tensor_tensor(tmpi[:], psi[:], S[:], OP.mult)
                    F = pw.tile([128, TPC], F32, name="F")
                    nc.scalar.activation(F[:], tmpf[:], AF.Sigmoid, scale=rwsf)
                    G = pw.tile([128, TPC], F32, name="G")
                    nc.scalar.activation(G[:], tmpf[:], AF.Sigmoid, scale=rwsfn)
                    sil = pw.tile([128, TPC], F32, name="tmpf")
                    nc.scalar.activation(sil[:], tmpi[:], AF.Silu, scale=rwsi)
                    Iin = pw.tile([128, TPC], F32, name="tmpi")
                    nc.vector.tensor_tensor(Iin[:], sil[:], G[:], OP.mult)
                    hm = hsB[:, m * TPC:(m + 1) * TPC]
                    nc.vector.tensor_tensor_scan(hm, F[:], Iin[:], 0.0,
                                                 OP.mult, OP.add)
                    nc.vector.tensor_tensor_scan(
                        fcB[:, m * TPC:(m + 1) * TPC], F[:], F[:], 1.0,
                        OP.mult, OP.bypass)
                    nc.vector.tensor_copy(bnd[:, m:m + 1], hm[:, TPC - 1:TPC])

            # =============== Phase C: carry exchange (async) ===============
            nc.vector.tensor_scalar_mul(bnd2[:], bnd[:], me[:])
            cin = dram.tile([128, MT], F32)
            cout = dram.tile([128, MT], F32)
            nc.sync.dma_start(cin[:], bnd2[:])
            nc.gpsimd.collective_compute(
                "AllReduce", OP.add,
                replica_groups=[[0, 1], [2, 3], [4, 5], [6, 7]],
                ins=[cin.opt()], outs=[cout.opt()],
            )
            nc.sync.dma_start(carry_sb[:], cout[:])
            nc.vector.tensor_scalar_mul(carried[:], carry_sb[:], mo[:])

            # ======= Phase G: g projection (covers the collective wait) =======
            rstd_g = rp.tile([1, TPC], F32)    # 1/sqrt(mean g^2 + eps)
            gsum_r = rp.tile([1, TPC], F32)    # sum g^2
            gsc_d3 = gsc_d[:].rearrange("p (m t) -> p m t", m=MT)
            with (
                tc.tile_pool(name="wgp", bufs=2) as wgp,
                tc.tile_pool(name="gst", bufs=3) as gst,
                tc.tile_pool(name="gw2", bufs=3) as gw2,
                tc.tile_pool(name="psg", bufs=2, space="PSUM") as psgp,
                tc.tile_pool(name="psq", bufs=1, space="PSUM") as psqp,
            ):
                for ch in range(GCH):
                    cs = ch * 512
                    psq = psqp.tile([1, 512], F32, name=f"psq{ch}")
                    for m in range(MT):
                        wm = wgp.tile([128, KT * 128], BF16, name="wg_m")
                        nc.sync.dma_start(
                            wm[:], wgt_d.ap()[:, m * KT * 128:(m + 1) * KT * 128])
                        psg = psgp.tile([128, 512], F32)
                        for k in range(KT):
                            nc.tensor.matmul(psg[:], wm[:, k * 128:(k + 1) * 128],
                                             xqT[:, k * TPC + cs:k * TPC + cs + 512],
                                             start=(k == 0), stop=(k == KT - 1))
                        gsb = gst.tile([128, 512], F16)
                        nc.vector.scalar_tensor_tensor(
                            gsb[:], psg[:], gwsc[:, m:m + 1], S[:, cs:cs + 512],
                            OP.mult, OP.mult)
                        nc.sync.dma_start(gsc_d3[:, m, cs:cs + 512], gsb[:])
                        gsq = gw2.tile([128, 512], BF16)
                        nc.scalar.activation(gsq[:], gsb[:], AF.Square)
                        nc.tensor.matmul(psq[:], ones128[:], gsq[:],
                                         start=(m == 0), stop=(m == MT - 1))
                    nc.vector.tensor_copy(gsum_r[:, cs:cs + 512], psq[:])
                # rstd_g row
                stdg = rp.tile([1, TPC], F32)
                nc.scalar.activation(stdg[:], gsum_r[:], AF.Sqrt,
                                     bias=epsb[0:1, :], scale=1.0 / D)
                nc.vector.reciprocal(rstd_g[:], stdg[:])

            xq_ctx.__exit__(None, None, None)

            # =============== Phase F: carry fixup (2 half-chunks) ===============
            for half in range(2):
                h0, h1 = half * 512, (half + 1) * 512
                for m in range(MT):
                    hm = hsB[:, m * TPC + h0:m * TPC + h1]
                    nc.vector.scalar_tensor_tensor(
                        hm, fcB[:, m * TPC + h0:m * TPC + h1],
                        carried[:, m:m + 1], hm, OP.mult, OP.add)

            fc_ctx.__exit__(None, None, None)

            # =============== Phase T: gate + o-proj, chunked ===============
            dq_r = rp.tile([1, TPC], F32)      # out-drain scale per token
            hsB3 = hsB[:].rearrange("p (m t) -> p m t", m=MT)
            with (
                tc.tile_pool(name="gcl", bufs=2) as gcl,
                tc.tile_pool(name="gt", bufs=1) as gt,
                tc.tile_pool(name="dqp", bufs=2) as dqp,
                tc.tile_pool(name="oq", bufs=2) as oqp,
                tc.tile_pool(name="wos", bufs=2) as wos,
                tc.tile_pool(name="ot", bufs=4) as ot,
                tc.tile_pool(name="pso", bufs=2, space="PSUM") as psop,
                tc.tile_pool(name="psb", bufs=2, space="PSUM") as psbp,
            ):
                oqcs = [None] * OCH
                dqbs = [None] * OCH

                def gate_chunk(ch):
                    cs = ch * OCW
                    hv = hsB3[:, :, cs:cs + OCW]
                    gc = gcl.tile([128, MT * OCW], F16)
                    gc3 = gc[:].rearrange("p (m t) -> p m t", m=MT)
                    nc.sync.dma_start(gc3[:, :, :], gsc_d3[:, :, cs:cs + OCW])
                    hsig = gt.tile([128, MT * OCW], F32, name="hsig")
                    hsig3 = hsig[:].rearrange("p (m t) -> p m t", m=MT)
                    nc.scalar.activation(hsig3[:, :, :], hv, AF.Silu)
                    t_c = gt.tile([128, MT * OCW], F32, name="t_c")
                    t3 = t_c[:].rearrange("p (m t) -> p m t", m=MT)
                    nc.vector.tensor_tensor(t_c[:], gc[:], hsig[:], OP.mult)
                    # per-token sumsq via ones-matmul on bf16 squares
                    tsq = gt.tile([128, MT * OCW], BF16, name="tsq")
                    nc.scalar.activation(tsq[:], t_c[:], AF.Square)
                    tsq3 = tsq[:].rearrange("p (m t) -> p m t", m=MT)
                    psq = psbp.tile([1, OCW], F32, name="psqo")
                    for m in range(MT):
                        nc.tensor.matmul(psq[:], ones128[:], tsq3[:, m, :],
                                         start=(m == 0), stop=(m == MT - 1))
                    # per-token absmax: max over m on DVE, partitions on gpsimd
                    mxm = gt.tile([128, OCW], F32, name="mxm")
                    nc.vector.tensor_reduce(
                        mxm[:], t_c[:].rearrange("p (m t) -> p t m", m=MT),
                        mybir.AxisListType.X, OP.max, apply_absolute_value=True)
                    mxb = gt.tile([128, OCW], F32, name="mxb")
                    nc.gpsimd.partition_all_reduce(mxb[:], mxm[:], 128,
                                                   bass_isa.ReduceOp.max)
                    csb = gt.tile([128, OCW], F32, name="csb")
                    nc.vector.reciprocal(csb[:], mxb[:])
                    # u = t * (1/max|t|); round(127*u) to int; cast to bf16
                    u = gt.tile([128, MT * OCW], F32, name="hsig")
                    u3 = u[:].rearrange("p (m t) -> p m t", m=MT)
                    nc.vector.tensor_tensor(
                        u3[:, :, :], t3[:, :, :],
                        csb[:].unsqueeze(1).broadcast_to([128, MT, OCW]),
                        OP.mult)
                    uq = gt.tile([128, MT * OCW], I32, name="uq")
                    nc.scalar.activation(uq[:], u[:], AF.Identity, scale=127.0)
                    oqc = oqp.tile([128, MT * OCW], BF16)
                    nc.vector.tensor_copy(oqc[:], uq[:])
                    oqcs[ch] = oqc

                    # ---- row algebra for out-drain scale ----
                    rg_c = rstd_g[:, cs:cs + OCW]
                    o2 = rp.tile([1, OCW], F32, name="o2")
                    nc.vector.tensor_tensor(o2[:], psq[:], rg_c, OP.mult)
                    nc.vector.tensor_tensor(o2[:], o2[:], rg_c, OP.mult)
                    stdo = rp.tile([1, OCW], F32, name="stdo")
                    nc.scalar.activation(stdo[:], o2[:], AF.Sqrt,
                                         bias=epsb[0:1, :], scale=1.0 / D)
                    rstdo = rp.tile([1, OCW], F32, name="rstdo")
                    nc.vector.reciprocal(rstdo[:], stdo[:])
                    # max|on| = rgb * rstd_o * max|t| ; dq = clip/127 * rwso
                    mon = rp.tile([1, OCW], F32, name="mon")
                    nc.vector.tensor_tensor(mon[:], mxb[0:1, :], rg_c, OP.mult)
                    nc.vector.tensor_tensor(mon[:], mon[:], rstdo[:], OP.mult)
                    nc.vector.tensor_scalar_max(mon[:], mon[:], EPS)
                    nc.vector.tensor_scalar(dq_r[:, cs:cs + OCW], mon[:],
                                            1.0 / 127.0, rws[0:1, 3:4],
                                            OP.mult, OP.mult)
                    # broadcast dq to [128, OCW]
                    pdq = psbp.tile([128, OCW], F32, name="pdq")
                    nc.tensor.matmul(pdq[:], ones1[:], dq_r[:, cs:cs + OCW],
                                     start=True, stop=True)
                    dqb = dqp.tile([128, OCW], F32)
                    nc.scalar.copy(dqb[:], pdq[:])
                    dqbs[ch] = dqb

                def o_pass(c0, c1):
                    oq0 = oqcs[c0][:].rearrange("p (k t) -> p k t", k=KT)
                    oq1 = oqcs[c1][:].rearrange("p (k t) -> p k t", k=KT)
                    for m in range(MT):
                        wo_m = wos.tile([128, KT * 128], BF16, name="wo_m")
                        nc.sync.dma_start(
                            wo_m[:],
                            wot_d.ap()[:, m * KT * 128:(m + 1) * KT * 128])
                        pso0 = psop.tile([128, OCW], F32, name="pso0")
                        pso1 = psop.tile([128, OCW], F32, name="pso1")
                        for k in range(KT):
                            wk = wo_m[:, k * 128:(k + 1) * 128]
                            st, sp = (k == 0), (k == KT - 1)
                            nc.tensor.matmul(pso0[:], wk, oq0[:, k, :],
                                             start=st, stop=sp)
                            nc.tensor.matmul(pso1[:], wk, oq1[:, k, :],
                                             start=st, stop=sp)
                        for ch, pso in ((c0, pso0), (c1, pso1)):
                            om = ot.tile([128, OCW], F32)
                            nc.vector.tensor_tensor(om[:], pso[:],
                                                    dqbs[ch][:], OP.mult)
                            nc.sync.dma_start(
                                out_d.ap()[m * 128:(m + 1) * 128,
                                           ch * OCW:(ch + 1) * OCW], om[:])

                gate_chunk(0)
                gate_chunk(1)
                o_pass(0, 1)
                gate_chunk(2)
                gate_chunk(3)
                o_pass(2, 3)

    nc.compile()
    return nc


_NC_CACHE = None
LAST_RESULTS = None


def _get_nc():
    global _NC_CACHE
    if _NC_CACHE is None:
        _NC_CACHE = build_nc()
    return _NC_CACHE


def _quant_weight(w):
    """fla BitLinear ternary weight quant. w [out, in] f32.
    Returns integer-valued f32 WT [in, out] and the reciprocal scale 1/ws."""
    import jax
    import jax.numpy as jnp

    mean_abs = np.asarray(
        jax.jit(lambda a: jnp.mean(jnp.abs(a)), backend="cpu")(w)
    )
    ws = np.float32(1.0) / np.maximum(mean_abs.astype(np.float32), np.float32(1e-5))
    wq = np.clip(np.round(w * ws), -1.0, 1.0).astype(np.float32)
    return wq.T.copy(), np.float32(1.0) / ws


def _tile_fm(wt):
    """WT [d_in, d_out] -> [128p, (m k c)] with [p,m,k,c] = WT[k*128+p, m*128+c]."""
    return np.ascontiguousarray(
        wt.reshape(KT, 128, MT, 128).transpose(1, 2, 0, 3).reshape(128, -1)
    ).astype(ml_dtypes.bfloat16)


def kernel(hidden_states, Wi, Wf, Wg, Wo, g_norm_weight):
    nc = _get_nc()

    wiq, rwsi = _quant_weight(np.asarray(Wi))
    wfq, rwsf = _quant_weight(np.asarray(Wf))
    wgq, rwsg = _quant_weight(np.asarray(Wg))
    woq, rwso = _quant_weight(np.asarray(Wo))

    wit = _tile_fm(wiq).reshape(128, MT, KT * 128)
    wft = _tile_fm(wfq).reshape(128, MT, KT * 128)
    wift = np.ascontiguousarray(
        np.stack([wit, wft], axis=2).reshape(128, -1))
    wgt = _tile_fm(wgq)
    wot = _tile_fm(woq)

    gw = np.asarray(g_norm_weight, dtype=np.float32).reshape(MT, 128)
    gwsc = np.ascontiguousarray(gw.T * rwsg)   # [128, MT]
    x = np.asarray(hidden_states, dtype=np.float32)

    in_maps = []
    for c in range(NCORES):
        b, half = c // 2, c % 2
        rw = np.zeros((128, 4), np.float32)
        rw[:, 0] = rwsi
        rw[:, 1] = rwsf
        rw[:, 2] = -rwsf
        rw[:, 3] = rwso
        in_maps.append({
            "x": np.ascontiguousarray(x[b, half * TPC:(half + 1) * TPC, :]),
            "wift": wift, "wgt": wgt, "wot": wot, "gwsc": gwsc,
            "mask_even": np.full((128, 1), 1.0 - half, np.float32),
            "mask_odd": np.full((128, 1), float(half), np.float32),
            "rws": rw,
        })

    import os
    trace = bool(os.environ.get("HGRN_TRACE"))
    res = run_bass_kernel_spmd(nc, in_maps, list(range(NCORES)), trace=trace)
    global LAST_RESULTS
    LAST_RESULTS = res
    out = np.empty((B, L, D), np.float32)
    for c in range(NCORES):
        b, half = c // 2, c % 2
        out[b, half * TPC:(half + 1) * TPC, :] = res.results[c]["out"].T
    return out
